# revision 12
# baseline (speedup 1.0000x reference)
"""CenterLoss Trainium2 kernel (Bass/Tile, 8 NeuronCores, SPMD).

Math: for x[B,F], labels[B], centers[C,F] the reference computes
    distmat = ||x||^2 + ||c||^2 - 2 x @ c.T          # [B, C]
    loss = sum(clip(distmat * onehot(labels), 1e-12, 1e12)) / B
The one-hot mask keeps exactly one entry per row (distmat[i, labels[i]]);
every other entry is exactly 0.0 and clips to 1e-12.  So
    loss = (sum_i clip(||x_i - c_{l_i}||^2, 1e-12, 1e12)
            + (B*C - B) * 1e-12) / B
which needs a 128-row gather of centers per core instead of the full
[B, C] distmat (64 KB of table reads per core instead of 6.4 MB).

Sharding: batch split 128 rows per core; centers replicated (each core's
indirect DMA reads only the rows its labels select).  Per core the device
computes clip(||x_i - c_{l_i}||^2); the host sums the 8x128 partials, adds
the clip constant for the B*C-B zero entries, and divides by B.

Device dataflow per core (one pass):
  1. one DMA loads xi = [labels bitcast to f32 | -x]  (col 0 | cols 1..F)
  2. indirect DMA gathers centers[l_p] into a separate tile (bypass; the
     RMW-add variant costs ~0.9us more on HW than the extra DVE op)
  3. two DVE scalar_tensor_tensor ops: diff = c + (-x), then square +
     row-reduce, accum into column 0 of a zeroed [128,32] tile
  4. StreamTranspose the [128,32] tile: the 128 distances land as 32
     contiguous floats in each of partitions {0,32,64,96}.  A [128,1]
     partition-strided store costs ~8.5us on HW (128 4-byte partition
     reads); the transposed 4-line store costs ~1.5us.
  5. DMA out [4,32]
Rows are pre-sorted by label on the host (pure permutation; the final sum
is permutation-invariant) so the gather walks the table monotonically.
The clip of the 1024 distances happens in combine() on the host.
"""
import numpy as np

import concourse.bass as bass
import concourse.bacc as bacc
import concourse.tile as tile
from concourse import mybir
from concourse.bass_utils import run_bass_kernel_spmd

BATCH, NUM_CLASS, FEAT = 1024, 100000, 128
N_CORES = 8
ROWS = BATCH // N_CORES  # 128 rows per core, one SBUF partition each

_NC_CACHE = {}


def _emit_body(nc, sb, xi_d, cen_d, out_ap):
    xi = sb.tile([ROWS, FEAT + 1], mybir.dt.float32)
    ct = sb.tile([ROWS, FEAT], mybir.dt.float32)
    sq = sb.tile([ROWS, FEAT], mybir.dt.float32)
    d32 = sb.tile([ROWS, 32], mybir.dt.float32)
    t32 = sb.tile([ROWS, 32], mybir.dt.float32)
    # zero d32 on DVE; overlaps the DMAs (only col 0 gets real data, but
    # the transpose reads all 32 cols, which must be finite)
    nc.vector.memset(d32[:], 0.0)
    nc.sync.dma_start(out=xi[:], in_=xi_d[:, :])
    # gather centers[labels[p], :] into ct.  bypass, not RMW-add onto
    # -x: the cce read-modify-write costs ~0.9us extra on HW, more than
    # the extra DVE subtract below
    nc.gpsimd.indirect_dma_start(
        out=ct[:], out_offset=None,
        in_=cen_d[:, :],
        in_offset=bass.IndirectOffsetOnAxis(
            ap=xi[:, :1].bitcast(mybir.dt.int32), axis=0),
        compute_op=mybir.AluOpType.bypass)
    # sq = c + (-x);  then d[p] = sum_f sq^2 accumulated into col 0 of
    # d32 (ct doubles as scratch for the squared values)
    nc.vector.scalar_tensor_tensor(
        out=sq[:], in0=ct[:], scalar=1.0,
        in1=xi[:, 1:FEAT + 1], op0=mybir.AluOpType.mult,
        op1=mybir.AluOpType.add)
    nc.vector.scalar_tensor_tensor(
        out=ct[:], in0=sq[:], scalar=1.0,
        in1=sq[:], op0=mybir.AluOpType.mult,
        op1=mybir.AluOpType.mult, accum_out=d32[:, :1])
    # 32x32 block transpose: d32[32b+i, 0] -> t32[32b, i], so the 128
    # distances become 4 contiguous 32-float partition lines.  A [128,1]
    # partition-strided store costs ~8.5us; this 4-line store ~1.5us.
    nc.vector.transpose(out=t32[:], in_=d32[:])
    # single_packet shaves ~0.2us off the 4-line store
    nc.sync.dma_start(out=out_ap, in_=t32[0:ROWS:32, 0:32],
                      single_packet=True)


def build_nc():
    """The graded single-shot SPMD program (cached)."""
    if "main" in _NC_CACHE:
        return _NC_CACHE["main"]
    nc = bacc.Bacc("TRN2", target_bir_lowering=False, debug=False,
                   num_devices=N_CORES)
    xi_d = nc.dram_tensor("xi", [ROWS, FEAT + 1], mybir.dt.float32,
                          kind="ExternalInput").ap()
    cen_d = nc.dram_tensor("centers", [NUM_CLASS, FEAT], mybir.dt.float32,
                           kind="ExternalInput").ap()
    out_d = nc.dram_tensor("out", [ROWS // 32, 32], mybir.dt.float32,
                           kind="ExternalOutput").ap()
    with tile.TileContext(nc) as tc:
        with tc.tile_pool(name="sb", bufs=1) as sb:
            _emit_body(nc, sb, xi_d, cen_d, out_d[:, :])
    nc.compile()
    _NC_CACHE["main"] = nc
    return nc


def build_nc_timing(n_iters):
    """For_i-amplified variant for HW timing (centers internal: same DMA
    pattern, garbage values, so the 51MB table isn't staged per call)."""
    key = ("loop", n_iters)
    if key in _NC_CACHE:
        return _NC_CACHE[key]
    nc = bacc.Bacc("TRN2", target_bir_lowering=False, debug=False,
                   num_devices=N_CORES)
    xi_d = nc.dram_tensor("xi", [ROWS, FEAT + 1], mybir.dt.float32,
                          kind="ExternalInput").ap()
    cen_d = nc.dram_tensor("centers", [NUM_CLASS, FEAT],
                           mybir.dt.float32).ap()
    out_d = nc.dram_tensor("out", [ROWS // 32, 32], mybir.dt.float32,
                           kind="ExternalOutput").ap()
    with tile.TileContext(nc) as tc:
        with tc.tile_pool(name="sb", bufs=1) as sb:
            with tc.For_i(0, n_iters, 1):
                _emit_body(nc, sb, xi_d, cen_d, out_d[:, :])
    nc.compile()
    _NC_CACHE[key] = nc
    return nc


def make_in_maps(x, labels, centers):
    x = np.ascontiguousarray(x, dtype=np.float32)
    centers = np.ascontiguousarray(centers, dtype=np.float32)
    labels = np.asarray(labels).astype(np.int32).reshape(BATCH)
    in_maps = []
    for k in range(N_CORES):
        sl = slice(k * ROWS, (k + 1) * ROWS)
        ls, xs = labels[sl], x[sl]
        order = np.argsort(ls)  # permutation only; sum is order-invariant
        xi = np.empty((ROWS, FEAT + 1), dtype=np.float32)
        xi[:, 0] = ls[order].view(np.float32)
        xi[:, 1:] = -xs[order]
        in_maps.append({"xi": xi, "centers": centers})
    return in_maps


def combine(partials):
    clipped = np.clip(partials, 1e-12, 1e12)
    loss = (np.sum(clipped, dtype=np.float64)
            + (BATCH * NUM_CLASS - BATCH) * 1e-12) / BATCH
    return np.asarray(loss, dtype=np.float32)


def kernel(x, labels, centers):
    nc = build_nc()
    in_maps = make_in_maps(x, labels, centers)
    res = run_bass_kernel_spmd(nc, in_maps, list(range(N_CORES)))
    partials = np.concatenate(
        [res.results[k]["out"].reshape(ROWS) for k in range(N_CORES)])
    return combine(partials)

